# revision 1
# baseline (speedup 1.0000x reference)
"""Trainium2 Bass kernel for a diffusers-style cross-attention block.

Problem (hardcoded shapes):
    hidden_states         [2, 2048, 1280] f32
    encoder_hidden_states [2, 2048, 1024] f32
    Wq [1280, 1280]  Wk/Wv [1024, 1280]  Wo [1280, 1280]  b_o [1280]  (all f32)
    out = softmax((x Wq) (enc Wk)^T / 8) (enc Wv) Wo + b_o      (20 heads x 64)

Sharding across 8 NeuronCores: data-parallel on batch (2) x tensor-parallel on
heads (4 groups of 5 heads). Each core computes a partial output
[2048, 1280] = A_local @ Wo_rows for its 5 heads; the host sums the 4 partials
per batch element and adds the bias.

Per-core layout trick: the host passes TRANSPOSED activations (x^T, enc^T,
bf16), so Q^T and K^T come straight out of the projection matmuls, scores are
computed as S^T (kv on partitions, q on free), exp runs on the scalar engine
PSUM->SBUF, and the PV matmul consumes P^T directly with V stored naturally
[kv, d]. A ones-column appended to V makes the PV matmul also emit the softmax
denominator l[q]. No on-chip transposes anywhere.

Head pairs are packed into the 128 partitions (rows 0-63 / 64-127) and their
score matmuls are emitted back-to-back: lhsT base partitions 0/64 lower to PE
tile_position (0,0)/(64,0), so the two K=64 matmuls run concurrently in
disjoint row-groups of the systolic array.
"""

import numpy as np
import ml_dtypes
from contextlib import ExitStack

S = 2048          # seq len (q and kv)
C = 1280          # hidden
CC = 1024         # encoder hidden
HG = 5            # heads per core
D = 64            # head dim
HD = HG * D       # 320
VW = D + 1        # V columns incl. ones column
CK = C // 128     # 10
CCK = CC // 128   # 8
NKV = S // 128    # 16
NQ = S // 512     # 4

_CACHED = {}

# scheduling knobs (sweepable; defaults = best known)
CONFIG = {
    "phat_bufs": 16,
    "osb_bufs": 3,
    "small_bufs": 2,
    "f1_group": 3, "f2_group": 2, "og_group": 2,
    "rate0": 3, "rate1": 2, "rate2": 3,
}


def _emit(ctx, tc, xT, encT, wq, wk, wv, wo, out):
    from concourse import mybir

    nc = tc.nc
    bf16, f32 = mybir.dt.bfloat16, mybir.dt.float32
    Exp = mybir.ActivationFunctionType.Exp

    const = ctx.enter_context(tc.tile_pool(name="const", bufs=1))
    acts = ctx.enter_context(tc.tile_pool(name="acts", bufs=1))
    small = ctx.enter_context(tc.tile_pool(name="small", bufs=CONFIG["small_bufs"]))
    osb_pool = ctx.enter_context(tc.tile_pool(name="osb", bufs=CONFIG["osb_bufs"]))
    phat_pool = ctx.enter_context(tc.tile_pool(name="phat", bufs=CONFIG["phat_bufs"]))
    psum = ctx.enter_context(tc.tile_pool(name="psum", bufs=2, space="PSUM"))

    # ---- DMA in, critical-path order: wk, enc^T, wq, x^T, wv, wo ----
    wk_sb = const.tile([128, CCK * HD], bf16, tag="wk")
    nc.sync.dma_start(wk_sb[:], wk.rearrange("(k p) d -> p k d", p=128))
    encT_big = acts.tile([128, CCK * S], bf16, tag="encT")
    for c0, c1 in ((0, 4), (4, 8)):
        nc.sync.dma_start(
            encT_big[:, c0 * S:c1 * S],
            encT[c0 * 128:c1 * 128, :].rearrange("(k p) s -> p k s", p=128),
        )
    encT_sb = [encT_big[:, k * S:(k + 1) * S] for k in range(CCK)]
    wq_sb = const.tile([128, CK * HD], bf16, tag="wq")
    nc.sync.dma_start(wq_sb[:], wq.rearrange("(k p) d -> p k d", p=128))
    xT_big = acts.tile([128, CK * S], bf16, tag="xT")
    for c0, c1 in ((0, 4), (4, 8), (8, 10)):
        nc.sync.dma_start(
            xT_big[:, c0 * S:c1 * S],
            xT[c0 * 128:c1 * 128, :].rearrange("(k p) s -> p k s", p=128),
        )
    xT_sb = [xT_big[:, k * S:(k + 1) * S] for k in range(CK)]
    wv_sb = const.tile([128, CCK * HD], bf16, tag="wv")
    nc.sync.dma_start(wv_sb[:], wv.rearrange("(k p) d -> p k d", p=128))
    wo_sb = []
    for t in range(3):
        K = 128 if t < 2 else 64
        w = const.tile([128, C], bf16, tag=f"wo{t}", name=f"wo{t}")
        nc.sync.dma_start(w[:K, :], wo[t * 128:t * 128 + K, :])
        wo_sb.append(w)

    # persistent intermediates (head pairs packed into 128 partitions)
    qt_sb = [acts.tile([128, S], bf16, tag=f"qt{t}", name=f"qt{t}") for t in range(3)]
    kt_sb = [acts.tile([128, S], bf16, tag=f"kt{t}", name=f"kt{t}") for t in range(3)]
    at_sb = [acts.tile([128, S], bf16, tag=f"at{t}", name=f"at{t}") for t in range(3)]
    v_sb = acts.tile([128, NKV * HG * VW], bf16, tag="v")
    nc.vector.memset(v_sb[:], 1.0)  # ones columns; V blocks overwritten below

    def proj_qk_steps(w_sb, src_sb, nk, dst, t, group):
        """Generator: emits the Q/K projection for tile t in ~group-MM slices."""
        M = 128 if t < 2 else 64
        for j in range(NQ):
            ps = psum.tile([128, 512], f32, tag=("s" if j % 2 == 0 else "fill"),
                           name="ps", bufs=2)
            for k in range(nk):
                nc.tensor.matmul(
                    ps[:M, :],
                    lhsT=w_sb[:, k * HD + t * 128: k * HD + t * 128 + M],
                    rhs=src_sb[k][:, j * 512:(j + 1) * 512],
                    start=(k == 0), stop=(k == nk - 1),
                )
                if (k + 1) % group == 0:
                    yield
            nc.vector.tensor_copy(dst[:M, j * 512:(j + 1) * 512], ps[:M, :])
            yield

    def proj_qk(w_sb, src_sb, nk, dst, t):
        for _ in proj_qk_steps(w_sb, src_sb, nk, dst, t, group=999):
            pass

    def proj_v_tile(i):
        # one kv-tile of the V projection (+ ones column layout in v_sb)
        ps = psum.tile([128, 512], f32, tag="fill", name="ps", bufs=2)
        for k in range(CCK):
            nc.tensor.matmul(
                ps[:, :HD],
                lhsT=encT_sb[k][:, i * 128:(i + 1) * 128],
                rhs=wv_sb[:, k * HD:(k + 1) * HD],
                start=(k == 0), stop=(k == CCK - 1),
            )
        for h in range(HG):
            nc.vector.tensor_copy(
                v_sb[:, (i * HG + h) * VW: (i * HG + h) * VW + D],
                ps[:, h * D:(h + 1) * D],
            )

    def proj_qk_fill_steps(w_sb, src_sb, nk, dst, t, group):
        M = 128 if t < 2 else 64
        for j in range(NQ):
            ps = psum.tile([128, 512], f32, tag="fill", name="ps", bufs=2)
            for k in range(nk):
                nc.tensor.matmul(
                    ps[:M, :],
                    lhsT=w_sb[:, k * HD + t * 128: k * HD + t * 128 + M],
                    rhs=src_sb[k][:, j * 512:(j + 1) * 512],
                    start=(k == 0), stop=(k == nk - 1),
                )
                if (k + 1) % group == 0:
                    yield
            nc.vector.tensor_copy(dst[:M, j * 512:(j + 1) * 512], ps[:M, :])
            yield

    def proj_t2_pair_steps(group):
        """Q-t2 and K-t2 projections zipped, col-paired in the PE array:
        Q accumulates into PSUM partitions 0-63 (tile_position (0,0)), K into
        64-127 ((0,64)) so the two M=64 chains run in disjoint col-groups."""
        for j in range(NQ):
            psq = psum.tile([128, 512], f32, tag="fill", name="psq", bufs=2)
            psk = psum.tile([128, 512], f32, tag="fill", name="psk", bufs=2)
            cnt = 0
            for k in range(CK):
                nc.tensor.matmul(
                    psq[0:64, :],
                    lhsT=wq_sb[:, k * HD + 256: k * HD + 320],
                    rhs=xT_sb[k][:, j * 512:(j + 1) * 512],
                    start=(k == 0), stop=(k == CK - 1),
                )
                if k < CCK:
                    nc.tensor.matmul(
                        psk[64:128, :],
                        lhsT=wk_sb[:, k * HD + 256: k * HD + 320],
                        rhs=encT_sb[k][:, j * 512:(j + 1) * 512],
                        start=(k == 0), stop=(k == CCK - 1),
                    )
                cnt += 1
                if cnt % group == 0:
                    yield
            nc.vector.tensor_copy(qt_sb[2][0:64, j * 512:(j + 1) * 512], psq[0:64, :])
            nc.vector.tensor_copy(kt_sb[2][0:64, j * 512:(j + 1) * 512], psk[64:128, :])
            yield

    def outproj_steps(ms, group):
        for m in ms:
            osb = osb_pool.tile([128, C], f32, tag="osb", name="osb")
            cnt = 0
            for c0 in range(0, C, 512):
                cn = min(512, C - c0)
                ps = psum.tile([128, 512], f32, tag="fill", name="ops", bufs=2)
                for t in range(3):
                    K = 128 if t < 2 else 64
                    nc.tensor.matmul(
                        ps[:, :cn],
                        lhsT=at_sb[t][:K, m * 128:(m + 1) * 128],
                        rhs=wo_sb[t][:K, c0:c0 + cn],
                        start=(t == 0), stop=(t == 2),
                    )
                    cnt += 1
                    if cnt % group == 0:
                        yield
                nc.vector.tensor_copy(osb[:, c0:c0 + cn], ps[:, :cn])
            nc.sync.dma_start(out[m * 128:(m + 1) * 128, :], osb[:])
            yield

    def attention(t, v_interleave=False, fills=(None, None, None, None),
                  fill_rate=2):
        """Four 512-wide q-block rounds; fills[r] is a generator driven during
        round r (must only read data produced in rounds < r). Pair rounds pack
        both heads side-by-side in one [128,1024] score tile -> single exp."""
        heads = (2 * t, 2 * t + 1) if t < 2 else (4,)
        nh = len(heads)
        for jb in range(NQ):
            fill = fills[jb]
            pv = {}
            for h in heads:
                pv[h] = psum.tile([128, 512], f32, tag="pv", name="pv", bufs=2)
            # pair rounds pack (headA | headB) per kv-tile; single-head
            # rounds pack (kv-tile i | kv-tile i+1) -- one exp per 1024 cols
            istep = 2 // nh
            for i0 in range(0, NKV, istep):
                sps = psum.tile([128, 1024], f32, tag="s", name="sps", bufs=2)
                for sx in range(2):
                    h = heads[sx % nh]
                    i = i0 + sx // nh
                    rb0 = (h % 2) * 64
                    nc.tensor.matmul(
                        sps[:, sx * 512:(sx + 1) * 512],
                        lhsT=kt_sb[t][rb0:rb0 + 64, i * 128:(i + 1) * 128],
                        rhs=qt_sb[t][rb0:rb0 + 64, jb * 512:(jb + 1) * 512],
                        start=True, stop=True,
                    )
                ph = phat_pool.tile([128, 1024], bf16, tag="ph", name="ph")
                nc.scalar.activation(ph[:], sps[:], Exp, scale=0.125)
                if v_interleave and jb == 0:
                    proj_v_tile(i0)
                if fill is not None:
                    for _ in range(fill_rate):
                        next(fill, None)
                for sx in range(2):
                    h = heads[sx % nh]
                    i = i0 + sx // nh
                    nc.tensor.matmul(
                        pv[h][:VW, :],
                        lhsT=v_sb[:, (i * HG + h) * VW: (i * HG + h + 1) * VW],
                        rhs=ph[:, sx * 512:(sx + 1) * 512],
                        start=(i == 0), stop=(i == NKV - 1),
                    )
            for h in heads:
                rb0 = (h % 2) * 64
                ov = small.tile([VW, 512], f32, tag="ov", name="ov")
                nc.vector.tensor_copy(ov[:], pv[h][:VW, :])  # frees the PV slot
                r2 = small.tile([1, 512], f32, tag="r2", name="r2")
                nc.vector.reciprocal(r2[:], ov[64:65, :])
                rb = small.tile([64, 512], f32, tag="rb", name="rb")
                nc.gpsimd.partition_broadcast(rb[:], r2[:])
                nc.vector.tensor_mul(
                    at_sb[t][rb0:rb0 + 64, jb * 512:(jb + 1) * 512],
                    ov[0:64, :], rb[:],
                )
            if fill is not None:
                next(fill, None)

    def chain(*gens):
        for g in gens:
            yield from g

    def drain(g):
        for _ in g:
            pass

    fill1 = chain(proj_qk_fill_steps(wk_sb, encT_sb, CCK, kt_sb[1], 1, CONFIG["f1_group"]),
                  proj_qk_fill_steps(wq_sb, xT_sb, CK, qt_sb[1], 1, CONFIG["f1_group"]))
    fill2 = proj_t2_pair_steps(CONFIG["f2_group"])

    proj_qk(wk_sb, encT_sb, CCK, kt_sb[0], 0)
    proj_qk(wq_sb, xT_sb, CK, qt_sb[0], 0)
    attention(0, v_interleave=True, fills=(None, fill1, fill1, fill1),
              fill_rate=CONFIG["rate0"])
    drain(fill1)
    attention(1, fills=(fill2, fill2, fill2, fill2), fill_rate=CONFIG["rate1"])
    drain(fill2)
    # out-projection row-group k (m = 4k..4k+3) reads at columns produced by
    # round k, so it may only be driven in rounds > k.
    og = [outproj_steps(range(4 * k, 4 * k + 4), CONFIG["og_group"]) for k in range(3)]
    attention(2, fills=(None, og[0], og[1], og[2]), fill_rate=CONFIG["rate2"])
    for g in og:
        drain(g)
    drain(outproj_steps(range(12, NKV), 999))


def build():
    if "nc" in _CACHED:
        return _CACHED["nc"]
    import concourse.tile as tile
    from concourse import bacc, mybir

    bf16, f32 = mybir.dt.bfloat16, mybir.dt.float32
    nc = bacc.Bacc("TRN2", target_bir_lowering=False, debug=False)
    xT = nc.dram_tensor("xT", [C, S], bf16, kind="ExternalInput").ap()
    encT = nc.dram_tensor("encT", [CC, S], bf16, kind="ExternalInput").ap()
    wq = nc.dram_tensor("wq", [C, HD], bf16, kind="ExternalInput").ap()
    wk = nc.dram_tensor("wk", [CC, HD], bf16, kind="ExternalInput").ap()
    wv = nc.dram_tensor("wv", [CC, HD], bf16, kind="ExternalInput").ap()
    wo = nc.dram_tensor("wo", [HD, C], bf16, kind="ExternalInput").ap()
    out = nc.dram_tensor("out", [S, C], f32, kind="ExternalOutput").ap()

    with tile.TileContext(nc) as tc:
        with ExitStack() as ctx:
            _emit(ctx, tc, xT, encT, wq, wk, wv, wo, out)
    nc.compile()
    _CACHED["nc"] = nc
    return nc


def make_in_maps(hidden_states, encoder_hidden_states, Wq, Wk, Wv, Wo):
    bf = ml_dtypes.bfloat16
    in_maps = []
    xTs = [np.ascontiguousarray(hidden_states[b].T).astype(bf) for b in range(2)]
    encTs = [np.ascontiguousarray(encoder_hidden_states[b].T).astype(bf) for b in range(2)]
    for core in range(8):
        b, g = divmod(core, 4)
        cols = slice(g * HD, (g + 1) * HD)
        in_maps.append({
            "xT": xTs[b],
            "encT": encTs[b],
            "wq": np.ascontiguousarray(Wq[:, cols]).astype(bf),
            "wk": np.ascontiguousarray(Wk[:, cols]).astype(bf),
            "wv": np.ascontiguousarray(Wv[:, cols]).astype(bf),
            "wo": np.ascontiguousarray(Wo[cols, :]).astype(bf),
        })
    return in_maps


def kernel(hidden_states, encoder_hidden_states, Wq, Wk, Wv, Wo, b_o):
    from concourse.bass_utils import run_bass_kernel_spmd

    nc = build()
    in_maps = make_in_maps(hidden_states, encoder_hidden_states, Wq, Wk, Wv, Wo)
    res = run_bass_kernel_spmd(nc, in_maps, core_ids=list(range(8)))
    outs = [res.results[c]["out"] for c in range(8)]
    full = np.stack([
        outs[0] + outs[1] + outs[2] + outs[3],
        outs[4] + outs[5] + outs[6] + outs[7],
    ]).astype(np.float32)
    full += np.asarray(b_o, np.float32)
    return full



# revision 6
# speedup vs baseline: 1.0162x; 1.0162x over previous
"""Trainium2 Bass kernel for a diffusers-style cross-attention block.

Problem (hardcoded shapes):
    hidden_states         [2, 2048, 1280] f32
    encoder_hidden_states [2, 2048, 1024] f32
    Wq [1280, 1280]  Wk/Wv [1024, 1280]  Wo [1280, 1280]  b_o [1280]  (all f32)
    out = softmax((x Wq) (enc Wk)^T / 8) (enc Wv) Wo + b_o      (20 heads x 64)

Sharding across 8 NeuronCores: data-parallel on batch (2) x tensor-parallel on
heads (4 groups of 5 heads). Each core computes a partial output
[2048, 1280] = A_local @ Wo_rows for its 5 heads; the host sums the 4 partials
per batch element and adds the bias.

Kernel structure (per core):
  - Q/K/V projections run as fp8 DoubleRow matmuls with hi/lo residual
    splitting: W = e4m3(W) + e5m2 residual, x likewise; three accumulation
    chains (W8*x8, Wr*x8, W8*xr) recover ~bf16 accuracy at half the
    per-column PE cost, contracting 256 rows per instruction.
  - Scores are computed transposed, S^T[kv, q], in bf16 (128-col q windows).
  - exp runs on the scalar engine (its only job), PSUM -> SBUF bf16.
  - PV emits A[q, 65] per head (65-wide instructions; the 65th V column is
    ones so the softmax denominator falls out of the same matmul).
  - Normalization = DVE reciprocal + per-partition tensor_scalar muls.
  - A tiles are transposed via the DMA XBAR (dma_start_transpose) into the
    head-pair-packed A^T layout the output projection consumes.
  - Output projection accumulates in PSUM and DMAs straight from PSUM to
    DRAM f32.
"""

import numpy as np
import ml_dtypes
from contextlib import ExitStack

S = 2048          # seq len (q and kv)
C = 1280          # hidden
CC = 1024         # encoder hidden
HG = 5            # heads per core
D = 64            # head dim
HD = HG * D       # 320
VW = D + 1        # V columns incl. ones column
CK = C // 128     # 10
CCK = CC // 128   # 8
NKV = S // 128    # 16 kv tiles
NW = S // 128     # 16 q windows
QB = S // 512     # 4 q/kv 512-blocks

_CACHED = {}

CONFIG = {
    "warm": 48,        # PE warm-up matmuls (N=128 each) during initial DMA
    "ph_bufs": 6,
    "rate": 3,         # fill ops driven per (head, kv-group) slot
    "w0_rate": 10,     # fill rate inside window 0 (heavy V/K fills)
}


def _emit(ctx, tc):
    from concourse import mybir

    nc = tc.nc
    bf16, f32 = mybir.dt.bfloat16, mybir.dt.float32
    e4, e5 = mybir.dt.float8e4, mybir.dt.float8e5
    DR = mybir.MatmulPerfMode.DoubleRow
    Exp = mybir.ActivationFunctionType.Exp

    # ---- DRAM tensors ----
    dram = {}
    for nm, shape, dt in [
        ("x8", [128, CK * S], e4), ("xr", [128, CK * S], e5),
        ("enc8", [128, CCK * S], e4), ("encr", [128, CCK * S], e5),
        ("wq8", [128, CK * HD], e4), ("wqr", [128, CK * HD], e5),
        ("wk8", [128, CCK * HD], e4), ("wkr", [128, CCK * HD], e5),
        ("wv8", [128, CCK * HD], e4), ("wvr", [128, CCK * HD], e5),
        ("wo", [HD, C], bf16),
    ]:
        dram[nm] = nc.dram_tensor(nm, shape, dt, kind="ExternalInput").ap()
    out = nc.dram_tensor("out", [S, C], bf16, kind="ExternalOutput").ap()

    const = ctx.enter_context(tc.tile_pool(name="const", bufs=1))
    acts = ctx.enter_context(tc.tile_pool(name="acts", bufs=1))
    ph_pool = ctx.enter_context(tc.tile_pool(name="php", bufs=CONFIG["ph_bufs"]))
    osb_pool = ctx.enter_context(tc.tile_pool(name="osbp", bufs=3))
    psum = ctx.enter_context(tc.tile_pool(name="psum", bufs=2, space="PSUM"))

    # ---- SBUF tiles ----
    sb = {}
    for nm, shape, dt in [
        ("x8", [128, CK * S], e4), ("xr", [128, CK * S], e5),
        ("enc8", [128, CCK * S], e4), ("encr", [128, CCK * S], e5),
        ("wq8", [128, CK * HD], e4), ("wqr", [128, CK * HD], e5),
        ("wk8", [128, CCK * HD], e4), ("wkr", [128, CCK * HD], e5),
        ("wv8", [128, CCK * HD], e4), ("wvr", [128, CCK * HD], e5),
    ]:
        sb[nm] = acts.tile(shape, dt, tag=nm, name=nm)
    wo_sb = []
    for t in range(3):
        K = 128 if t < 2 else 64
        w = const.tile([128, C], bf16, tag=f"wo{t}", name=f"wo{t}")
        wo_sb.append(w)
    qt = [acts.tile([128, S], bf16, tag=f"qt{t}", name=f"qt{t}") for t in range(3)]
    kt = [acts.tile([128, S], bf16, tag=f"kt{t}", name=f"kt{t}") for t in range(3)]
    at = [acts.tile([128, S], bf16, tag=f"at{t}", name=f"at{t}") for t in range(3)]
    v_sb = acts.tile([128, NKV * HG * VW], bf16, tag="v", name="v_sb")
    a_t = [acts.tile([128, 384], bf16, tag=f"a{i}", name=f"a{i}") for i in range(2)]
    rl_t = [acts.tile([128, HG], f32, tag=f"rl{i}", name=f"rl{i}") for i in range(2)]

    # ---- memsets (DVE) ----
    nc.vector.memset(v_sb[:], 1.0)      # ones columns; V blocks overwritten
    for i in range(2):
        nc.vector.memset(a_t[i][:, HD:384], 0.0)  # transpose pad

    # ---- DMA in (SP queue), earliest-needed first ----
    def dma_w(nm):
        nc.sync.dma_start(sb[nm][:], dram[nm])

    def dma_act(nm, nk, b):
        # 512-col block b of a [128, nk, S] activation tensor
        sv = sb[nm][:].rearrange("p (c s) -> p c s", c=nk)[:, :, b * 512:(b + 1) * 512]
        dv = dram[nm].rearrange("p (c s) -> p c s", c=nk)[:, :, b * 512:(b + 1) * 512]
        nc.sync.dma_start(sv, dv)

    dma_w("wk8"); dma_w("wkr")
    dma_act("enc8", CCK, 0); dma_act("encr", CCK, 0)
    dma_w("wq8"); dma_w("wqr")
    dma_act("x8", CK, 0); dma_act("xr", CK, 0)
    dma_act("enc8", CCK, 1); dma_act("encr", CCK, 1)
    dma_w("wv8"); dma_w("wvr")
    for b in range(2, 4):
        dma_act("enc8", CCK, b); dma_act("encr", CCK, b)
    for b in range(1, 4):
        dma_act("x8", CK, b); dma_act("xr", CK, b)
    for t in range(3):
        K = 128 if t < 2 else 64
        nc.sync.dma_start(wo_sb[t][:K, :], dram["wo"][t * 128:t * 128 + K, :])

    # 3D views for DoubleRow chains
    x8v = sb["x8"][:].rearrange("p (c s) -> p c s", c=CK)
    xrv = sb["xr"][:].rearrange("p (c s) -> p c s", c=CK)
    e8v = sb["enc8"][:].rearrange("p (c s) -> p c s", c=CCK)
    erv = sb["encr"][:].rearrange("p (c s) -> p c s", c=CCK)
    wq8v = sb["wq8"][:].rearrange("p (c m) -> p c m", c=CK)
    wqrv = sb["wqr"][:].rearrange("p (c m) -> p c m", c=CK)
    wk8v = sb["wk8"][:].rearrange("p (c m) -> p c m", c=CCK)
    wkrv = sb["wkr"][:].rearrange("p (c m) -> p c m", c=CCK)
    wv8v = sb["wv8"][:].rearrange("p (c m) -> p c m", c=CCK)
    wvrv = sb["wvr"][:].rearrange("p (c m) -> p c m", c=CCK)

    # ---- PE warm-up: keep PE busy (and ramping) during initial DMA ----
    warm_ps = psum.tile([128, 512], f32, tag="blk", name="warm", bufs=2)
    for _ in range(CONFIG["warm"]):
        nc.tensor.matmul(warm_ps[:, 0:128], lhsT=v_sb[:, 0:128],
                         rhs=v_sb[:, 0:128], start=True, stop=True)

    # ---- projection emitters (fp8 DoubleRow, 3 residual chains) ----
    def proj_qk_block_steps(w8, wr, xv8, xvr, nk, dst, b):
        """dst[:, b*512:(b+1)*512] for all 3 row-regions, one region per yield
        group. nk = number of 128-row contraction chunks (10 for Q, 8 for K)."""
        np_ = nk // 2
        for t in range(3):
            M = 128 if t < 2 else 64
            ps = psum.tile([128, 512], f32, tag="blk", name="pblk", bufs=2)
            first, last = (0, 0), (2, np_ - 1)
            cn = 0
            for ci, (wv, xv) in enumerate(((w8, xv8), (wr, xv8), (w8, xvr))):
                for p in range(np_):
                    nc.tensor.matmul(
                        ps[:M, :],
                        lhsT=wv[:, 2 * p:2 * p + 2, t * 128:t * 128 + M],
                        rhs=xv[:, 2 * p:2 * p + 2, b * 512:(b + 1) * 512],
                        start=(ci, p) == first, stop=(ci, p) == last,
                        perf_mode=DR,
                    )
                    cn += 1
                    if cn % 4 == 0:
                        yield
            nc.vector.tensor_copy(dst[t][:M, b * 512:(b + 1) * 512], ps[:M, :])
            yield

    def vproj_tile_steps(i):
        """V projection for kv-tile i -> v_sb (ones col at 64 of each 65)."""
        ps = psum.tile([128, 512], f32, tag="blk", name="vblk", bufs=2)
        first, last = (0, 0), (2, CCK // 2 - 1)
        cn = 0
        for ci, (lv, wv) in enumerate(((e8v, wv8v), (erv, wv8v), (e8v, wvrv))):
            for p in range(CCK // 2):
                nc.tensor.matmul(
                    ps[:, :HD],
                    lhsT=lv[:, 2 * p:2 * p + 2, i * 128:(i + 1) * 128],
                    rhs=wv[:, 2 * p:2 * p + 2, :],
                    start=(ci, p) == first, stop=(ci, p) == last,
                    perf_mode=DR,
                )
                cn += 1
                if cn % 4 == 0:
                    yield
        vdst = v_sb[:, i * HG * VW:(i + 1) * HG * VW].rearrange(
            "p (h w) -> p h w", h=HG)[:, :, 0:D]
        nc.vector.tensor_copy(vdst, ps[:, :HD].rearrange("p (h w) -> p h w", h=HG))
        yield

    def oproj_steps(m):
        """Output projection for q-tile m: A^T[:, m*128:+128] @ Wo -> out."""
        for c0 in range(0, C, 512):
            cn = min(512, C - c0)
            ps = psum.tile([128, 512], f32, tag="blk", name="oblk", bufs=2)
            for t in range(3):
                K = 128 if t < 2 else 64
                nc.tensor.matmul(
                    ps[:, :cn],
                    lhsT=at[t][:K, m * 128:(m + 1) * 128],
                    rhs=wo_sb[t][:K, c0:c0 + cn],
                    start=(t == 0), stop=(t == 2),
                )
                yield
            osb = osb_pool.tile([128, 512], bf16, tag="osb", name="osb")
            nc.vector.tensor_copy(osb[:, :cn], ps[:, :cn])
            nc.sync.dma_start(out[m * 128:(m + 1) * 128, c0:c0 + cn], osb[:, :cn])
            yield

    # ---- fill driver ----
    fills = []

    def drive(n):
        while fills and n > 0:
            try:
                next(fills[0])
                n -= 1
            except StopIteration:
                fills.pop(0)

    def drain(g):
        for _ in g:
            pass

    def drain_all():
        while fills:
            drain(fills.pop(0))

    # ---- phase A: K blocks 0-1, Q block 0 (direct) ----
    drain(proj_qk_block_steps(wk8v, wkrv, e8v, erv, CCK, kt, 0))
    drain(proj_qk_block_steps(wk8v, wkrv, e8v, erv, CCK, kt, 1))
    drain(proj_qk_block_steps(wq8v, wqrv, x8v, xrv, CK, qt, 0))

    # ---- main attention loop: 16 q-windows of 128 ----
    def head_tiles(h):
        if h < 4:
            return kt[h // 2], qt[h // 2], 64 * (h % 2)
        return kt[2], qt[2], 0

    for jb in range(NW):
        pv = psum.tile([128, 512], f32, tag="pv", name="pv", bufs=2)
        for g in range(2):
            for h in range(HG):
                ktt, qtt, rb = head_tiles(h)
                sps = psum.tile([128, 1024], f32, tag="s", name="sps", bufs=2)
                for j in range(8):
                    i = 8 * g + j
                    nc.tensor.matmul(
                        sps[:, j * 128:(j + 1) * 128],
                        lhsT=ktt[rb:rb + 64, i * 128:(i + 1) * 128],
                        rhs=qtt[rb:rb + 64, jb * 128:(jb + 1) * 128],
                        start=(j % 4 == 0), stop=(j % 4 == 3),
                    )
                ph = ph_pool.tile([128, 1024], bf16, tag="ph", name="ph")
                nc.scalar.activation(ph[:], sps[:], Exp, scale=0.125)
                if jb == 0:
                    if g == 0 and h == 0:
                        for i in range(8):
                            drain(vproj_tile_steps(i))
                    else:
                        drive(CONFIG["w0_rate"])
                else:
                    drive(CONFIG["rate"])
                for j in range(8):
                    i = 8 * g + j
                    nc.tensor.matmul(
                        pv[:, h * VW:(h + 1) * VW],
                        lhsT=ph[:, j * 128:(j + 1) * 128],
                        rhs=v_sb[:, (i * HG + h) * VW:(i * HG + h + 1) * VW],
                        start=(g == 0 and h == 0 and j == 0),
                        stop=(g == 1 and h == HG - 1 and j == 7),
                    )
            if jb == 0 and g == 0:
                # V tiles 8-15 and K blocks 2,3 must land before g=1 reads them
                for i in range(8, 16):
                    fills.append(vproj_tile_steps(i))
                fills.append(proj_qk_block_steps(wk8v, wkrv, e8v, erv, CCK, kt, 2))
                fills.append(proj_qk_block_steps(wk8v, wkrv, e8v, erv, CCK, kt, 3))
                drain_all()
        # normalize + transpose this window's A
        a = a_t[jb % 2]
        rl = rl_t[jb % 2]
        nc.vector.reciprocal(rl[:], pv[:, D:HG * VW:VW])
        for h in range(HG):
            nc.vector.tensor_scalar_mul(
                a[:, h * D:(h + 1) * D], pv[:, h * VW:h * VW + D], rl[:, h:h + 1])
        nc.sync.dma_start_transpose(at[0][:, jb * 128:(jb + 1) * 128], a[:, 0:128])
        nc.sync.dma_start_transpose(at[1][:, jb * 128:(jb + 1) * 128], a[:, 128:256])
        nc.sync.dma_start_transpose(at[2][:, jb * 128:(jb + 1) * 128], a[:, 256:384])
        fills.append(oproj_steps(jb))
        if jb in (1, 5, 9):
            fills.append(proj_qk_block_steps(wq8v, wqrv, x8v, xrv, CK, qt,
                                             jb // 4 + 1))
    drain_all()


def build():
    if "nc" in _CACHED:
        return _CACHED["nc"]
    import concourse.tile as tile
    from concourse import bacc

    nc = bacc.Bacc("TRN2", target_bir_lowering=False, debug=False)
    with tile.TileContext(nc) as tc:
        with ExitStack() as ctx:
            _emit(ctx, tc)
    nc.compile()
    _CACHED["nc"] = nc
    return nc


def _split85(a):
    """f32 array -> (e4m3 main, e5m2 residual)."""
    hi = a.astype(ml_dtypes.float8_e4m3)
    lo = (a - hi.astype(np.float32)).astype(ml_dtypes.float8_e5m2)
    return hi, lo


def _act_layout(aT, nk):
    """[nk*128, S] -> [128, nk*S] with chunk-major free dim."""
    return np.ascontiguousarray(
        aT.reshape(nk, 128, S).transpose(1, 0, 2).reshape(128, nk * S))


def _w_layout(w, nk):
    """[nk*128, HD] -> [128, nk*HD]."""
    return np.ascontiguousarray(
        w.reshape(nk, 128, HD).transpose(1, 0, 2).reshape(128, nk * HD))


def make_in_maps(hidden_states, encoder_hidden_states, Wq, Wk, Wv, Wo):
    bf = ml_dtypes.bfloat16
    xs, encs = [], []
    for b in range(2):
        xT = np.ascontiguousarray(np.asarray(hidden_states[b], np.float32).T)
        x8, xr = _split85(xT)
        xs.append((_act_layout(x8, CK), _act_layout(xr, CK)))
        eT = np.ascontiguousarray(np.asarray(encoder_hidden_states[b], np.float32).T)
        e8, er = _split85(eT)
        encs.append((_act_layout(e8, CCK), _act_layout(er, CCK)))
    in_maps = []
    for core in range(8):
        b, g = divmod(core, 4)
        cols = slice(g * HD, (g + 1) * HD)
        wq8, wqr = _split85(np.ascontiguousarray(np.asarray(Wq[:, cols], np.float32)))
        wk8, wkr = _split85(np.ascontiguousarray(np.asarray(Wk[:, cols], np.float32)))
        wv8, wvr = _split85(np.ascontiguousarray(np.asarray(Wv[:, cols], np.float32)))
        in_maps.append({
            "x8": xs[b][0], "xr": xs[b][1],
            "enc8": encs[b][0], "encr": encs[b][1],
            "wq8": _w_layout(wq8, CK), "wqr": _w_layout(wqr, CK),
            "wk8": _w_layout(wk8, CCK), "wkr": _w_layout(wkr, CCK),
            "wv8": _w_layout(wv8, CCK), "wvr": _w_layout(wvr, CCK),
            "wo": np.ascontiguousarray(np.asarray(Wo[cols, :], np.float32)).astype(bf),
        })
    return in_maps


def kernel(hidden_states, encoder_hidden_states, Wq, Wk, Wv, Wo, b_o):
    from concourse.bass_utils import run_bass_kernel_spmd

    nc = build()
    in_maps = make_in_maps(hidden_states, encoder_hidden_states, Wq, Wk, Wv, Wo)
    res = run_bass_kernel_spmd(nc, in_maps, core_ids=list(range(8)))
    outs = [np.asarray(res.results[c]["out"], np.float32) for c in range(8)]
    full = np.stack([
        outs[0] + outs[1] + outs[2] + outs[3],
        outs[4] + outs[5] + outs[6] + outs[7],
    ]).astype(np.float32)
    full += np.asarray(b_o, np.float32)
    return full


# revision 21
# speedup vs baseline: 1.1185x; 1.1007x over previous
"""Trainium2 Bass kernel for a diffusers-style cross-attention block.

Problem (hardcoded shapes):
    hidden_states         [2, 2048, 1280] f32
    encoder_hidden_states [2, 2048, 1024] f32
    Wq [1280, 1280]  Wk/Wv [1024, 1280]  Wo [1280, 1280]  b_o [1280]  (all f32)
    out = softmax((x Wq) (enc Wk)^T / 8) (enc Wv) Wo + b_o      (20 heads x 64)

Sharding across 8 NeuronCores: data-parallel on batch (2) x tensor-parallel on
heads (4 groups of 5 heads). Each core computes a partial output
[2048, 1280] = A_local @ Wo_rows for its 5 heads; the host sums the 4 partials
per batch element and adds the bias.

Kernel structure (per core):
  - Q/K/V projections run as fp8 DoubleRow matmuls with hi/lo residual
    splitting: W = e4m3(W) + e5m2 residual, x likewise; three accumulation
    chains (W8*x8, Wr*x8, W8*xr) recover ~bf16 accuracy at half the
    per-column PE cost, contracting 256 rows per instruction.
  - Scores are computed transposed, S^T[kv, q], in bf16 (128-col q windows).
  - exp runs on the scalar engine (its only job), PSUM -> SBUF bf16.
  - PV emits A[q, 65] per head (65-wide instructions; the 65th V column is
    ones so the softmax denominator falls out of the same matmul).
  - Normalization = DVE reciprocal + per-partition tensor_scalar muls.
  - A tiles are transposed via the DMA XBAR (dma_start_transpose) into the
    head-pair-packed A^T layout the output projection consumes.
  - Output projection accumulates in PSUM and DMAs straight from PSUM to
    DRAM f32.
"""

import numpy as np
import ml_dtypes
from contextlib import ExitStack

S = 2048          # seq len (q and kv)
C = 1280          # hidden
CC = 1024         # encoder hidden
HG = 5            # heads per core
D = 64            # head dim
HD = HG * D       # 320
VW = D + 1        # V columns incl. ones column
CK = C // 128     # 10
CCK = CC // 128   # 8
NKV = S // 128    # 16 kv tiles
NW = S // 128     # 16 q windows
QB = S // 512     # 4 q/kv 512-blocks

_CACHED = {}

CONFIG = {
    "warm": 36,        # PE warm-up matmuls (N=128 each) during initial DMA
    "ph_bufs": 12,
    "rate": 1,         # fill ops driven per (head, kv-group) slot
    "w01_rate": 8,     # fill rate inside windows 0/1 (heavy V/K fills)
}


def _emit(ctx, tc):
    from concourse import mybir

    nc = tc.nc
    bf16, f32 = mybir.dt.bfloat16, mybir.dt.float32
    e4, e5 = mybir.dt.float8e4, mybir.dt.float8e5
    DR = mybir.MatmulPerfMode.DoubleRow
    Exp = mybir.ActivationFunctionType.Exp

    # ---- DRAM tensors ----
    dram = {}
    for nm, shape, dt in [
        ("x8", [128, CK * S], e4), ("xr", [128, CK * S], e5),
        ("enc8", [128, CCK * S], e4), ("encr", [128, CCK * S], e5),
        ("wq8", [128, CK * HD], e4), ("wqr", [128, CK * HD], e5),
        ("wk8", [128, CCK * HD], e4), ("wkr", [128, CCK * HD], e5),
        ("wv8", [128, CCK * HD], e4), ("wvr", [128, CCK * HD], e5),
        ("wo", [HD, C], bf16),
    ]:
        dram[nm] = nc.dram_tensor(nm, shape, dt, kind="ExternalInput").ap()
    out = nc.dram_tensor("out", [S, C], bf16, kind="ExternalOutput").ap()

    const = ctx.enter_context(tc.tile_pool(name="const", bufs=1))
    acts = ctx.enter_context(tc.tile_pool(name="acts", bufs=1))
    ph_pool = ctx.enter_context(tc.tile_pool(name="php", bufs=CONFIG["ph_bufs"]))
    osb_pool = ctx.enter_context(tc.tile_pool(name="osbp", bufs=3))
    psum = ctx.enter_context(tc.tile_pool(name="psum", bufs=2, space="PSUM"))

    # ---- SBUF tiles ----
    sb = {}
    for nm, shape, dt in [
        ("x8", [128, CK * S], e4), ("xr", [128, CK * S], e5),
        ("enc8", [128, CCK * S], e4), ("encr", [128, CCK * S], e5),
        ("wq8", [128, CK * HD], e4), ("wqr", [128, CK * HD], e5),
        ("wk8", [128, CCK * HD], e4), ("wkr", [128, CCK * HD], e5),
        ("wv8", [128, CCK * HD], e4), ("wvr", [128, CCK * HD], e5),
    ]:
        sb[nm] = acts.tile(shape, dt, tag=nm, name=nm)
    wo_sb = []
    for t in range(3):
        K = 128 if t < 2 else 64
        w = const.tile([128, C], bf16, tag=f"wo{t}", name=f"wo{t}")
        wo_sb.append(w)
    qt = [acts.tile([128, S], bf16, tag=f"qt{t}", name=f"qt{t}") for t in range(3)]
    kt = [acts.tile([128, S], bf16, tag=f"kt{t}", name=f"kt{t}") for t in range(3)]
    at = [acts.tile([128, S], bf16, tag=f"at{t}", name=f"at{t}") for t in range(3)]
    v_sb = acts.tile([128, NKV * HG * VW], bf16, tag="v", name="v_sb")
    a_t = [acts.tile([128, 384], bf16, tag=f"a{i}", name=f"a{i}") for i in range(2)]
    rl_t = [acts.tile([128, HG], f32, tag=f"rl{i}", name=f"rl{i}") for i in range(2)]

    wdum = const.tile([128, 128], bf16, tag="wdum", name="wdum")

    # ---- memsets (DVE) ----
    nc.vector.memset(wdum[:], 0.0)      # warm-up matmul input, ready instantly
    v3 = v_sb[:].rearrange("p (i h w) -> p i h w", i=NKV, h=HG)
    nc.vector.memset(v3[:, :, :, D:VW], 1.0)   # ones columns only
    for i in range(2):
        nc.vector.memset(a_t[i][:, HD:384], 0.0)  # transpose pad

    # ---- DMA in (SP queue), earliest-needed first ----
    def dma_w(nm):
        nc.sync.dma_start(sb[nm][:], dram[nm])

    def dma_act(nm, nk, b):
        # 512-col block b of a [128, nk, S] activation tensor
        sv = sb[nm][:].rearrange("p (c s) -> p c s", c=nk)[:, :, b * 512:(b + 1) * 512]
        dv = dram[nm].rearrange("p (c s) -> p c s", c=nk)[:, :, b * 512:(b + 1) * 512]
        nc.sync.dma_start(sv, dv)

    dma_w("wk8"); dma_act("enc8", CCK, 0)
    dma_w("wkr"); dma_act("encr", CCK, 0)
    dma_act("enc8", CCK, 1); dma_act("encr", CCK, 1)
    dma_w("wq8"); dma_act("x8", CK, 0)
    dma_w("wqr"); dma_act("xr", CK, 0)
    dma_w("wv8"); dma_w("wvr")
    for b in range(2, 4):
        dma_act("enc8", CCK, b); dma_act("encr", CCK, b)
    for b in range(1, 4):
        dma_act("x8", CK, b); dma_act("xr", CK, b)
    for t in range(3):
        K = 128 if t < 2 else 64
        nc.sync.dma_start(wo_sb[t][:K, :], dram["wo"][t * 128:t * 128 + K, :])

    # 3D views for DoubleRow chains
    x8v = sb["x8"][:].rearrange("p (c s) -> p c s", c=CK)
    xrv = sb["xr"][:].rearrange("p (c s) -> p c s", c=CK)
    e8v = sb["enc8"][:].rearrange("p (c s) -> p c s", c=CCK)
    erv = sb["encr"][:].rearrange("p (c s) -> p c s", c=CCK)
    wq8v = sb["wq8"][:].rearrange("p (c m) -> p c m", c=CK)
    wqrv = sb["wqr"][:].rearrange("p (c m) -> p c m", c=CK)
    wk8v = sb["wk8"][:].rearrange("p (c m) -> p c m", c=CCK)
    wkrv = sb["wkr"][:].rearrange("p (c m) -> p c m", c=CCK)
    wv8v = sb["wv8"][:].rearrange("p (c m) -> p c m", c=CCK)
    wvrv = sb["wvr"][:].rearrange("p (c m) -> p c m", c=CCK)

    # ---- PE warm-up: keep PE busy (and ramping) during initial DMA ----
    warm_ps = psum.tile([128, 512], f32, tag="blk", name="warm", bufs=2)
    for _ in range(CONFIG["warm"]):
        nc.tensor.matmul(warm_ps[:, 0:128], lhsT=wdum[:],
                         rhs=wdum[:], start=True, stop=True)

    # ---- projection emitters (fp8 DoubleRow, 3 residual chains) ----
    def kq_region_steps(w8, wr, xv8, xvr, nk, dst, b, t):
        """One (512-col block b, row-region t) of a Q/K projection."""
        np_ = nk // 2
        M = 128 if t < 2 else 64
        ps = psum.tile([128, 512], f32, tag="blk", name="pblk", bufs=2)
        first, last = (0, 0), (2, np_ - 1)
        cn = 0
        for ci, (wv, xv) in enumerate(((w8, xv8), (wr, xv8), (w8, xvr))):
            for p in range(np_):
                nc.tensor.matmul(
                    ps[:M, :],
                    lhsT=wv[:, 2 * p:2 * p + 2, t * 128:t * 128 + M],
                    rhs=xv[:, 2 * p:2 * p + 2, b * 512:(b + 1) * 512],
                    start=(ci, p) == first, stop=(ci, p) == last,
                    perf_mode=DR,
                )
                cn += 1
                if cn % 4 == 0:
                    yield
        nc.vector.tensor_copy(dst[t][:M, b * 512:(b + 1) * 512], ps[:M, :])
        yield

    def chain(*gens):
        for g in gens:
            yield from g

    def vproj_tile_steps(i):
        """V projection for kv-tile i -> v_sb (ones col at 64 of each 65)."""
        ps = psum.tile([128, 512], f32, tag="blk", name="vblk", bufs=2)
        first, last = (0, 0), (2, CCK // 2 - 1)
        cn = 0
        for ci, (lv, wv) in enumerate(((e8v, wv8v), (erv, wv8v), (e8v, wvrv))):
            for p in range(CCK // 2):
                nc.tensor.matmul(
                    ps[:, :HD],
                    lhsT=lv[:, 2 * p:2 * p + 2, i * 128:(i + 1) * 128],
                    rhs=wv[:, 2 * p:2 * p + 2, :],
                    start=(ci, p) == first, stop=(ci, p) == last,
                    perf_mode=DR,
                )
                cn += 1
                if cn % 4 == 0:
                    yield
        vdst = v_sb[:, i * HG * VW:(i + 1) * HG * VW].rearrange(
            "p (h w) -> p h w", h=HG)[:, :, 0:D]
        nc.vector.tensor_copy(vdst, ps[:, :HD].rearrange("p (h w) -> p h w", h=HG))
        yield

    def oproj_steps(m):
        """Output projection for q-tile m: A^T[:, m*128:+128] @ Wo -> out."""
        for c0 in range(0, C, 512):
            cn = min(512, C - c0)
            ps = psum.tile([128, 512], f32, tag="blk", name="oblk", bufs=2)
            for t in range(3):
                K = 128 if t < 2 else 64
                nc.tensor.matmul(
                    ps[:, :cn],
                    lhsT=at[t][:K, m * 128:(m + 1) * 128],
                    rhs=wo_sb[t][:K, c0:c0 + cn],
                    start=(t == 0), stop=(t == 2),
                )
                yield
            osb = osb_pool.tile([128, 512], bf16, tag="osb", name="osb")
            nc.vector.tensor_copy(osb[:, :cn], ps[:, :cn])
            nc.sync.dma_start(out[m * 128:(m + 1) * 128, c0:c0 + cn], osb[:, :cn])
            yield

    # ---- fill driver ----
    fills = []

    def drive(n):
        while fills and n > 0:
            try:
                next(fills[0])
                n -= 1
            except StopIteration:
                fills.pop(0)

    def drain(g):
        for _ in g:
            pass

    def drain_until(g):
        """Drive fills (FIFO) until generator g is exhausted."""
        if g not in fills:
            return  # already drained by earlier drive() calls
        while fills:
            cur = fills[0]
            try:
                next(cur)
            except StopIteration:
                fills.pop(0)
                if cur is g:
                    return

    def drain_all():
        while fills:
            drain(fills.pop(0))

    def kq_region(b, t):
        return kq_region_steps(wk8v, wkrv, e8v, erv, CCK, kt, b, t)

    def qq_region(b, t):
        return kq_region_steps(wq8v, wqrv, x8v, xrv, CK, qt, b, t)

    def head_tiles(h):
        if h < 4:
            return kt[h // 2], qt[h // 2], 64 * (h % 2)
        return kt[2], qt[2], 0

    pv_cnt = {}

    def scores_exp(jb, h, g):
        ktt, qtt, rb = head_tiles(h)
        sps = psum.tile([128, 1024], f32, tag="s", name="sps", bufs=2)
        for j in range(8):
            i = 8 * g + j
            nc.tensor.matmul(
                sps[:, j * 128:(j + 1) * 128],
                lhsT=ktt[rb:rb + 64, i * 128:(i + 1) * 128],
                rhs=qtt[rb:rb + 64, jb * 128:(jb + 1) * 128],
                start=(j % 4 == 0), stop=(j % 4 == 3),
            )
        ph = ph_pool.tile([128, 1024], bf16, tag="ph", name="ph")
        nc.scalar.activation(ph[:], sps[:], Exp, scale=0.125)
        return ph

    def pv_accum(jb, h, g, ph, pv):
        for j in range(8):
            i = 8 * g + j
            n = pv_cnt[jb]
            nc.tensor.matmul(
                pv[:, h * VW:(h + 1) * VW],
                lhsT=ph[:, j * 128:(j + 1) * 128],
                rhs=v_sb[:, (i * HG + h) * VW:(i * HG + h + 1) * VW],
                start=(n == 0), stop=(n == 79),
            )
            pv_cnt[jb] = n + 1

    def norm_transpose_pair(jb, pv, pair):
        # pair 0: heads 0,1 -> at0; pair 1: heads 2,3 -> at1; pair 2: head 4 -> at2
        a = a_t[jb % 2]
        rl = rl_t[jb % 2]
        h0 = 2 * pair
        nh = 1 if pair == 2 else 2
        nc.vector.reciprocal(rl[:, h0:h0 + nh],
                             pv[:, h0 * VW + D:(h0 + nh) * VW:VW])
        for h in range(h0, h0 + nh):
            nc.vector.tensor_scalar_mul(
                a[:, h * D:(h + 1) * D], pv[:, h * VW:h * VW + D], rl[:, h:h + 1])
        nc.sync.dma_start_transpose(
            at[pair][:, jb * 128:(jb + 1) * 128],
            a[:, pair * 128:(pair + 1) * 128])

    # ---- windows 0+1, paired, interleaved with projection/V fills ----
    RW = CONFIG["w01_rate"]
    drain(chain(kq_region(0, 0), kq_region(1, 0)))
    drain(qq_region(0, 0))
    a1 = fills.append
    gA1 = chain(kq_region(0, 1), kq_region(1, 1), qq_region(0, 1))
    gA2 = chain(kq_region(0, 2), kq_region(1, 2), qq_region(0, 2))
    gV = [vproj_tile_steps(i) for i in range(16)]
    fills.extend([gA1, gA2] + gV[:8])
    pv01 = {}
    for w in (0, 1):
        pv01[w] = psum.tile([128, 512], f32, tag="pv", name="pv", bufs=2)
        pv_cnt[w] = 0
    phs = {}
    for w, h in ((0, 0), (0, 1), (1, 0), (1, 1)):
        phs[w, h] = scores_exp(w, h, 0)
        drive(RW)
    drain_until(gA1)
    for w, h in ((0, 2), (0, 3), (1, 2), (1, 3)):
        phs[w, h] = scores_exp(w, h, 0)
        drive(RW)
    drain_until(gA2)
    for w, h in ((0, 4), (1, 4)):
        phs[w, h] = scores_exp(w, h, 0)
        drive(RW)
    drain_until(gV[7])
    for w in (0, 1):
        for h in range(HG):
            pv_accum(w, h, 0, phs[w, h], pv01[w])
    gK2 = [kq_region(2, t) for t in range(3)]
    gK3 = [kq_region(3, t) for t in range(3)]
    fills.extend([gK2[0], gK3[0], gK2[1], gK3[1], gK2[2], gK3[2]] + gV[8:])
    drain_until(gK3[0])
    for w, h in ((0, 0), (0, 1), (1, 0), (1, 1)):
        phs[w, h] = scores_exp(w, h, 1)
        drive(RW)
    drain_until(gK3[1])
    for w, h in ((0, 2), (0, 3), (1, 2), (1, 3)):
        phs[w, h] = scores_exp(w, h, 1)
        drive(RW)
    drain_until(gK3[2])
    for w, h in ((0, 4), (1, 4)):
        phs[w, h] = scores_exp(w, h, 1)
        drive(RW)
    drain_until(gV[15])
    for w in (0, 1):
        for h in range(HG):
            pv_accum(w, h, 1, phs[w, h], pv01[w])
            if h in (1, 3, 4):
                norm_transpose_pair(w, pv01[w], (h - 1) // 2 + (h == 4))
        fills.append(oproj_steps(w))
    phs.clear()

    # ---- steady windows 2..15: PV trails scores by 2 slots so PV's
    # dependency (its exp) is already satisfied when PE dequeues it ----
    pend = []

    def pop_pv():
        jb2, h2, g2, ph2, pv2 = pend.pop(0)
        pv_accum(jb2, h2, g2, ph2, pv2)
        if g2 == 1 and h2 in (1, 3, 4):
            norm_transpose_pair(jb2, pv2, (h2 - 1) // 2 + (h2 == 4))
        if g2 == 1 and h2 == HG - 1:
            fills.append(oproj_steps(jb2))

    qgen = {b: [qq_region(b, t) for t in range(3)] for b in (1, 2, 3)}
    pvt = {}
    for jb in range(2, NW):
        if jb in (2, 6, 10):
            # enqueue next q-block's region fills two windows ahead of use
            fills.extend(qgen[jb // 4 + 1])
        pvt[jb] = psum.tile([128, 512], f32, tag="pv", name="pv", bufs=2)
        pv_cnt[jb] = 0
        rate = CONFIG["rate"] + (jb >= 12)
        for g in range(2):
            for h in range(HG):
                if jb >= 4 and jb % 4 == 0 and g == 0:
                    # first window of q-block jb//4: its region must be in SBUF
                    drain_until(qgen[jb // 4][min(h // 2, 2)])
                ph = scores_exp(jb, h, g)
                drive(rate)
                if len(pend) >= 2:
                    pop_pv()
                drive(rate)
                pend.append((jb, h, g, ph, pvt[jb]))
    while pend:
        pop_pv()
        drive(CONFIG["rate"])
    drain_all()


def build():
    if "nc" in _CACHED:
        return _CACHED["nc"]
    import concourse.tile as tile
    from concourse import bacc

    nc = bacc.Bacc("TRN2", target_bir_lowering=False, debug=False)
    with tile.TileContext(nc) as tc:
        with ExitStack() as ctx:
            _emit(ctx, tc)
    nc.compile()
    _CACHED["nc"] = nc
    return nc


def _split85(a):
    """f32 array -> (e4m3 main, e5m2 residual)."""
    hi = a.astype(ml_dtypes.float8_e4m3)
    lo = (a - hi.astype(np.float32)).astype(ml_dtypes.float8_e5m2)
    return hi, lo


def _act_layout(aT, nk):
    """[nk*128, S] -> [128, nk*S] with chunk-major free dim."""
    return np.ascontiguousarray(
        aT.reshape(nk, 128, S).transpose(1, 0, 2).reshape(128, nk * S))


def _w_layout(w, nk):
    """[nk*128, HD] -> [128, nk*HD]."""
    return np.ascontiguousarray(
        w.reshape(nk, 128, HD).transpose(1, 0, 2).reshape(128, nk * HD))


def make_in_maps(hidden_states, encoder_hidden_states, Wq, Wk, Wv, Wo):
    bf = ml_dtypes.bfloat16
    xs, encs = [], []
    for b in range(2):
        xT = np.ascontiguousarray(np.asarray(hidden_states[b], np.float32).T)
        x8, xr = _split85(xT)
        xs.append((_act_layout(x8, CK), _act_layout(xr, CK)))
        eT = np.ascontiguousarray(np.asarray(encoder_hidden_states[b], np.float32).T)
        e8, er = _split85(eT)
        encs.append((_act_layout(e8, CCK), _act_layout(er, CCK)))
    in_maps = []
    for core in range(8):
        b, g = divmod(core, 4)
        cols = slice(g * HD, (g + 1) * HD)
        wq8, wqr = _split85(np.ascontiguousarray(np.asarray(Wq[:, cols], np.float32)))
        wk8, wkr = _split85(np.ascontiguousarray(np.asarray(Wk[:, cols], np.float32)))
        wv8, wvr = _split85(np.ascontiguousarray(np.asarray(Wv[:, cols], np.float32)))
        in_maps.append({
            "x8": xs[b][0], "xr": xs[b][1],
            "enc8": encs[b][0], "encr": encs[b][1],
            "wq8": _w_layout(wq8, CK), "wqr": _w_layout(wqr, CK),
            "wk8": _w_layout(wk8, CCK), "wkr": _w_layout(wkr, CCK),
            "wv8": _w_layout(wv8, CCK), "wvr": _w_layout(wvr, CCK),
            "wo": np.ascontiguousarray(np.asarray(Wo[cols, :], np.float32)).astype(bf),
        })
    return in_maps


def kernel(hidden_states, encoder_hidden_states, Wq, Wk, Wv, Wo, b_o):
    from concourse.bass_utils import run_bass_kernel_spmd

    nc = build()
    in_maps = make_in_maps(hidden_states, encoder_hidden_states, Wq, Wk, Wv, Wo)
    res = run_bass_kernel_spmd(nc, in_maps, core_ids=list(range(8)))
    outs = [np.asarray(res.results[c]["out"], np.float32) for c in range(8)]
    full = np.stack([
        outs[0] + outs[1] + outs[2] + outs[3],
        outs[4] + outs[5] + outs[6] + outs[7],
    ]).astype(np.float32)
    full += np.asarray(b_o, np.float32)
    return full


# revision 38
# speedup vs baseline: 1.2647x; 1.1307x over previous
"""Trainium2 Bass kernel for a diffusers-style cross-attention block.

Problem (hardcoded shapes):
    hidden_states         [2, 2048, 1280] f32
    encoder_hidden_states [2, 2048, 1024] f32
    Wq [1280, 1280]  Wk/Wv [1024, 1280]  Wo [1280, 1280]  b_o [1280]  (all f32)
    out = softmax((x Wq) (enc Wk)^T / 8) (enc Wv) Wo + b_o      (20 heads x 64)

Sharding across 8 NeuronCores: data-parallel on batch (2) x tensor-parallel on
heads (4 groups of 5 heads). Each core computes a partial output
[2048, 1280] = A_local @ Wo_rows for its 5 heads; the host sums the 4 partials
per batch element and adds the bias.

Kernel structure (per core):
  - Q/K/V projections run as fp8 DoubleRow matmuls with hi/lo residual
    splitting: W = e4m3(W) + e5m2 residual, x likewise; three accumulation
    chains (W8*x8, Wr*x8, W8*xr) recover ~bf16 accuracy at half the
    per-column PE cost, contracting 256 rows per instruction.
  - Scores are computed transposed, S^T[kv, q], in bf16 (128-col q windows).
  - exp runs on the scalar engine (its only job), PSUM -> SBUF bf16.
  - PV emits A[q, 65] per head (65-wide instructions; the 65th V column is
    ones so the softmax denominator falls out of the same matmul).
  - Normalization = DVE reciprocal + per-partition tensor_scalar muls.
  - A tiles are transposed via the DMA XBAR (dma_start_transpose) into the
    head-pair-packed A^T layout the output projection consumes.
  - Output projection accumulates in PSUM and DMAs straight from PSUM to
    DRAM f32.
"""

import numpy as np
import ml_dtypes
from contextlib import ExitStack

S = 2048          # seq len (q and kv)
C = 1280          # hidden
CC = 1024         # encoder hidden
HG = 5            # heads per core
D = 64            # head dim
HD = HG * D       # 320
VW = D + 1        # V columns incl. ones column
CK = C // 128     # 10
CCK = CC // 128   # 8
NKV = S // 128    # 16 kv tiles
NW = S // 128     # 16 q windows
QB = S // 512     # 4 q/kv 512-blocks

_CACHED = {}

CONFIG = {
    "warm": 36,        # PE warm-up matmuls (N=128 each) during initial DMA
    "ph_bufs": 12,
    "rate": 2,         # fill ops driven per (head, kv-group) slot
    "oproj_late": 1,   # delay oproj fills past the DMA-transpose latency
    "w01_rate": 12,     # fill rate inside windows 0/1 (heavy V/K fills)
}


def _emit(ctx, tc):
    from concourse import mybir

    nc = tc.nc
    bf16, f32 = mybir.dt.bfloat16, mybir.dt.float32
    e4, e5 = mybir.dt.float8e4, mybir.dt.float8e5
    DR = mybir.MatmulPerfMode.DoubleRow
    Exp = mybir.ActivationFunctionType.Exp

    # ---- DRAM tensors ----
    dram = {}
    for nm, shape, dt in [
        ("x8", [128, CK * S], e4), ("xr", [128, CK * S], e5),
        ("enc8", [128, CCK * S], e4), ("encr", [128, CCK * S], e5),
        ("wq8", [128, CK * HD], e4), ("wqr", [128, CK * HD], e5),
        ("wk8", [128, CCK * HD], e4), ("wkr", [128, CCK * HD], e5),
        ("wv8", [128, CCK * HD], e4), ("wvr", [128, CCK * HD], e5),
        ("wo", [HD, C], bf16),
    ]:
        dram[nm] = nc.dram_tensor(nm, shape, dt, kind="ExternalInput").ap()
    out = nc.dram_tensor("out", [S, C], bf16, kind="ExternalOutput").ap()

    const = ctx.enter_context(tc.tile_pool(name="const", bufs=1))
    acts = ctx.enter_context(tc.tile_pool(name="acts", bufs=1))
    ph_pool = ctx.enter_context(tc.tile_pool(name="php", bufs=CONFIG["ph_bufs"]))
    osb_pool = ctx.enter_context(tc.tile_pool(name="osbp", bufs=3))
    psum = ctx.enter_context(tc.tile_pool(name="psum", bufs=2, space="PSUM"))

    # ---- SBUF tiles ----
    sb = {}
    for nm, shape, dt in [
        ("x8", [128, CK * S], e4), ("xr", [128, CK * S], e5),
        ("enc8", [128, CCK * S], e4), ("encr", [128, CCK * S], e5),
        ("wq8", [128, CK * HD], e4), ("wqr", [128, CK * HD], e5),
        ("wk8", [128, CCK * HD], e4), ("wkr", [128, CCK * HD], e5),
        ("wv8", [128, CCK * HD], e4), ("wvr", [128, CCK * HD], e5),
    ]:
        sb[nm] = acts.tile(shape, dt, tag=nm, name=nm)
    wo_sb = []
    for t in range(3):
        K = 128 if t < 2 else 64
        w = const.tile([128, C], bf16, tag=f"wo{t}", name=f"wo{t}")
        wo_sb.append(w)
    qt = [acts.tile([128, S], bf16, tag=f"qt{t}", name=f"qt{t}") for t in range(3)]
    kt = [acts.tile([128, S], bf16, tag=f"kt{t}", name=f"kt{t}") for t in range(3)]
    at = [acts.tile([128, S], bf16, tag=f"at{t}", name=f"at{t}") for t in range(3)]
    v_sb = acts.tile([128, NKV * HG * VW], bf16, tag="v", name="v_sb")
    a_t = [acts.tile([128, 384], bf16, tag=f"a{i}", name=f"a{i}") for i in range(2)]
    rl_t = [acts.tile([128, HG], f32, tag=f"rl{i}", name=f"rl{i}") for i in range(2)]

    wdum = const.tile([128, 128], bf16, tag="wdum", name="wdum")

    # ---- memsets (DVE) ----
    nc.vector.memset(wdum[:], 0.0)      # warm-up matmul input, ready instantly
    v3 = v_sb[:].rearrange("p (i h w) -> p i h w", i=NKV, h=HG)
    nc.vector.memset(v3[:, :, :, D:VW], 1.0)   # ones columns only
    for i in range(2):
        nc.vector.memset(a_t[i][:, HD:384], 0.0)  # transpose pad

    # ---- DMA in (SP queue), earliest-needed first ----
    def dma_w(nm):
        nc.sync.dma_start(sb[nm][:], dram[nm])

    def dma_act(nm, nk, b):
        dma_act2(nm, nk, b * 512, 512)

    def dma_act2(nm, nk, c0, cw):
        # columns [c0, c0+cw) of a [128, nk, S] activation tensor
        sv = sb[nm][:].rearrange("p (c s) -> p c s", c=nk)[:, :, c0:c0 + cw]
        dv = dram[nm].rearrange("p (c s) -> p c s", c=nk)[:, :, c0:c0 + cw]
        nc.sync.dma_start(sv, dv)

    dma_w("wk8"); dma_act("enc8", CCK, 0)
    dma_w("wkr"); dma_act("encr", CCK, 0)
    dma_w("wq8"); dma_act2("x8", CK, 0, 256)
    dma_w("wqr"); dma_act2("xr", CK, 0, 256)
    dma_w("wv8"); dma_w("wvr")
    dma_act("enc8", CCK, 1); dma_act("encr", CCK, 1)
    dma_act2("x8", CK, 256, 256); dma_act2("xr", CK, 256, 256)
    for b in range(2, 4):
        dma_act("enc8", CCK, b); dma_act("encr", CCK, b)
    for b in range(1, 4):
        dma_act("x8", CK, b); dma_act("xr", CK, b)
    for t in range(3):
        K = 128 if t < 2 else 64
        nc.sync.dma_start(wo_sb[t][:K, :], dram["wo"][t * 128:t * 128 + K, :])

    # 3D views for DoubleRow chains
    x8v = sb["x8"][:].rearrange("p (c s) -> p c s", c=CK)
    xrv = sb["xr"][:].rearrange("p (c s) -> p c s", c=CK)
    e8v = sb["enc8"][:].rearrange("p (c s) -> p c s", c=CCK)
    erv = sb["encr"][:].rearrange("p (c s) -> p c s", c=CCK)
    wq8v = sb["wq8"][:].rearrange("p (c m) -> p c m", c=CK)
    wqrv = sb["wqr"][:].rearrange("p (c m) -> p c m", c=CK)
    wk8v = sb["wk8"][:].rearrange("p (c m) -> p c m", c=CCK)
    wkrv = sb["wkr"][:].rearrange("p (c m) -> p c m", c=CCK)
    wv8v = sb["wv8"][:].rearrange("p (c m) -> p c m", c=CCK)
    wvrv = sb["wvr"][:].rearrange("p (c m) -> p c m", c=CCK)

    # ---- PE warm-up: keep PE busy (and ramping) during initial DMA ----
    warm_ps = psum.tile([128, 512], f32, tag="blk", name="warm", bufs=2)
    for _ in range(CONFIG["warm"]):
        nc.tensor.matmul(warm_ps[:, 0:128], lhsT=wdum[:],
                         rhs=wdum[:], start=True, stop=True)

    # ---- projection emitters (fp8 DoubleRow, 3 residual chains) ----
    def kq_region_steps(w8, wr, xv8, xvr, nk, dst, c0, cw, t):
        """Columns [c0, c0+cw) x row-region t of a Q/K projection."""
        np_ = nk // 2
        M = 128 if t < 2 else 64
        ps = psum.tile([128, 512], f32, tag="blk", name="pblk", bufs=2)
        first, last = (0, 0), (2, np_ - 1)
        cn = 0
        for ci, (wv, xv) in enumerate(((w8, xv8), (wr, xv8), (w8, xvr))):
            for p in range(np_):
                nc.tensor.matmul(
                    ps[:M, :cw],
                    lhsT=wv[:, 2 * p:2 * p + 2, t * 128:t * 128 + M],
                    rhs=xv[:, 2 * p:2 * p + 2, c0:c0 + cw],
                    start=(ci, p) == first, stop=(ci, p) == last,
                    perf_mode=DR,
                )
                cn += 1
                if cn % 4 == 0:
                    yield
        nc.vector.tensor_copy(dst[t][:M, c0:c0 + cw], ps[:M, :cw])
        yield

    def chain(*gens):
        for g in gens:
            yield from g

    def vproj_tile_steps(i):
        """V projection for kv-tile i -> v_sb (ones col at 64 of each 65)."""
        ps = psum.tile([128, 512], f32, tag="blk", name="vblk", bufs=2)
        first, last = (0, 0), (2, CCK // 2 - 1)
        cn = 0
        for ci, (lv, wv) in enumerate(((e8v, wv8v), (erv, wv8v), (e8v, wvrv))):
            for p in range(CCK // 2):
                nc.tensor.matmul(
                    ps[:, :HD],
                    lhsT=lv[:, 2 * p:2 * p + 2, i * 128:(i + 1) * 128],
                    rhs=wv[:, 2 * p:2 * p + 2, :],
                    start=(ci, p) == first, stop=(ci, p) == last,
                    perf_mode=DR,
                )
                cn += 1
                if cn % 4 == 0:
                    yield
        vdst = v_sb[:, i * HG * VW:(i + 1) * HG * VW].rearrange(
            "p (h w) -> p h w", h=HG)[:, :, 0:D]
        nc.vector.tensor_copy(vdst, ps[:, :HD].rearrange("p (h w) -> p h w", h=HG))
        yield

    def oproj_steps(m):
        """Output projection for q-tile m: A^T[:, m*128:+128] @ Wo -> out."""
        for ci, c0 in enumerate(range(0, C, 512)):
            cn = min(512, C - c0)
            ps = psum.tile([128, 512], f32, tag="blk", name="oblk", bufs=2)
            for t in range(3):
                K = 128 if t < 2 else 64
                nc.tensor.matmul(
                    ps[:, :cn],
                    lhsT=at[t][:K, m * 128:(m + 1) * 128],
                    rhs=wo_sb[t][:K, c0:c0 + cn],
                    start=(t == 0), stop=(t == 2),
                )
                yield
            osb = osb_pool.tile([128, 512], bf16, tag="osb", name="osb")
            nc.vector.tensor_copy(osb[:, :cn], ps[:, :cn])
            nc.sync.dma_start(out[m * 128:(m + 1) * 128, c0:c0 + cn], osb[:, :cn])
            yield

    # ---- fill driver ----
    fills = []
    done = set()

    def drive(n):
        while fills and n > 0:
            try:
                next(fills[0])
                n -= 1
            except StopIteration:
                done.add(id(fills.pop(0)))

    def drain(g):
        for _ in g:
            pass
        done.add(id(g))

    def drain_until(g):
        """Drain generator g only (cross-generator order is free: data
        dependencies are tracked per tile by the framework)."""
        if g not in fills:
            return  # already drained by earlier drive() calls
        fills.remove(g)
        drain(g)

    def drain_all():
        while fills:
            drain(fills.pop(0))

    def kq_region(b, t):
        return kq_region_steps(wk8v, wkrv, e8v, erv, CCK, kt, 512 * b, 512, t)

    def qq_region(c0, cw, t):
        return kq_region_steps(wq8v, wqrv, x8v, xrv, CK, qt, c0, cw, t)

    def head_tiles(h):
        if h < 4:
            return kt[h // 2], qt[h // 2], 64 * (h % 2)
        return kt[2], qt[2], 0

    pv_cnt = {}

    def scores_exp(jb, h, g):
        ktt, qtt, rb = head_tiles(h)
        sps = psum.tile([128, 1024], f32, tag="s", name="sps", bufs=2)
        for j in range(8):
            i = 8 * g + j
            nc.tensor.matmul(
                sps[:, j * 128:(j + 1) * 128],
                lhsT=ktt[rb:rb + 64, i * 128:(i + 1) * 128],
                rhs=qtt[rb:rb + 64, jb * 128:(jb + 1) * 128],
                start=(j % 4 == 0), stop=(j % 4 == 3),
            )
        ph = ph_pool.tile([128, 1024], bf16, tag="ph", name="ph")
        nc.scalar.activation(ph[:], sps[:], Exp, scale=0.125)
        return ph

    def pv_accum(jb, h, g, ph, pv):
        for j in range(8):
            i = 8 * g + j
            n = pv_cnt[jb]
            nc.tensor.matmul(
                pv[:, h * VW:(h + 1) * VW],
                lhsT=ph[:, j * 128:(j + 1) * 128],
                rhs=v_sb[:, (i * HG + h) * VW:(i * HG + h + 1) * VW],
                start=(n == 0), stop=(n == 79),
            )
            pv_cnt[jb] = n + 1

    def norm_transpose_pair(jb, pv, pair):
        # pair 0: heads 0,1 -> at0; pair 1: heads 2,3 -> at1; pair 2: head 4 -> at2
        a = a_t[jb % 2]
        rl = rl_t[jb % 2]
        h0 = 2 * pair
        nh = 1 if pair == 2 else 2
        nc.vector.reciprocal(rl[:, h0:h0 + nh],
                             pv[:, h0 * VW + D:(h0 + nh) * VW:VW])
        for h in range(h0, h0 + nh):
            nc.vector.tensor_scalar_mul(
                a[:, h * D:(h + 1) * D], pv[:, h * VW:h * VW + D], rl[:, h:h + 1])
        nc.sync.dma_start_transpose(
            at[pair][:, jb * 128:(jb + 1) * 128],
            a[:, pair * 128:(pair + 1) * 128])

    # ---- unified schedule: window-pairs x kv-quarters, head-pairs ----
    # Each pair-block (2wp, 2wp+1) runs 4 kv-quarters x 5 packed score
    # groups ([128,1024] = two (window,head) groups). PV pops trail the
    # score stream by >=3 slots (their exp is then provably complete), and
    # are additionally gated on the V tiles their quarter needs.
    def pairs_for(w0, w1):
        return (((w0, 0), (w0, 1)), ((w1, 0), (w1, 1)), ((w0, 2), (w0, 3)),
                ((w1, 2), (w1, 3)), ((w0, 4), (w1, 4)))

    PREG = (0, 0, 1, 1, 2)  # kt/qt row-region used by each pair index

    def scores_exp_q(pair, q):
        sps = psum.tile([128, 1024], f32, tag="s", name="sps", bufs=2)
        for sx, (w, h) in enumerate(pair):
            ktt, qtt, rb = head_tiles(h)
            for j in range(4):
                i = 4 * q + j
                nc.tensor.matmul(
                    sps[:, sx * 512 + j * 128:sx * 512 + (j + 1) * 128],
                    lhsT=ktt[rb:rb + 64, i * 128:(i + 1) * 128],
                    rhs=qtt[rb:rb + 64, w * 128:(w + 1) * 128],
                    start=(j == 0), stop=(j == 3),
                )
        ph = ph_pool.tile([128, 1024], bf16, tag="ph", name="ph")
        nc.scalar.activation(ph[:], sps[:], Exp, scale=0.125)
        return ph

    def pv_accum_q(pair, q, ph, pvd):
        for sx, (w, h) in enumerate(pair):
            for j in range(4):
                i = 4 * q + j
                n = pv_cnt[w]
                nc.tensor.matmul(
                    pvd[w][:, h * VW:(h + 1) * VW],
                    lhsT=ph[:, sx * 512 + j * 128:sx * 512 + (j + 1) * 128],
                    rhs=v_sb[:, (i * HG + h) * VW:(i * HG + h + 1) * VW],
                    start=(n == 0), stop=(n == 79),
                )
                pv_cnt[w] = n + 1

    pend = []

    def try_pops(budget, minlag=2):
        while pend and budget > 0 and len(pend) > minlag:
            pi, pair, q, ph2, pvd, wp2 = pend[0]
            if not all(id(gV[i]) in done for i in range(4 * q, 4 * q + 4)):
                return
            pend.pop(0)
            pv_accum_q(pair, q, ph2, pvd)
            budget -= 1
            if q == 3:
                w0, w1 = 2 * wp2, 2 * wp2 + 1
                if pi < 4:
                    norm_transpose_pair((w0, w1)[pi % 2], pvd[(w0, w1)[pi % 2]],
                                        pi // 2)
                else:
                    norm_transpose_pair(w0, pvd[w0], 2)
                    norm_transpose_pair(w1, pvd[w1], 2)
                    if wp2 == 7:
                        fills.append(oproj_steps(w0))
                        fills.append(oproj_steps(w1))
            elif q == 0 and wp2 >= 1 and pi in (1, 3):
                # previous pair's oproj, ~5 pops after its transposes began
                fills.append(oproj_steps(2 * wp2 - 2 + (pi == 3)))

    for t in range(3):
        drain(kq_region(0, t))
    gK0 = None
    QCOLS = ((0, 256), (256, 256), (512, 512), (1024, 512), (1536, 512))
    gQ = {k: [qq_region(c0, cw, t) for t in range(3)]
          for k, (c0, cw) in enumerate(QCOLS)}
    QBLK = (0, 1, 2, 2, 3, 3, 4, 4)  # pair-block -> q column-group
    gV = [vproj_tile_steps(i) for i in range(16)]
    gKb = {b: [kq_region(b, t) for t in range(3)] for b in (1, 2, 3)}
    fills.extend([gQ[0][0], gQ[0][1], gQ[0][2], gV[0], gV[1], gV[2], gV[3]]
                 + gKb[1] + gQ[1] + gV[4:8] + gKb[2] + gV[8:12]
                 + gKb[3] + gV[12:16])
    for wp in range(8):
        w0, w1 = 2 * wp, 2 * wp + 1
        pvd = {}
        for w in (w0, w1):
            pvd[w] = psum.tile([128, 512], f32, tag="pv", name="pv", bufs=2)
            pv_cnt[w] = 0
        if wp in (1, 2, 3):
            fills.extend(gQ[wp + 1])
        rate = CONFIG["w01_rate"] if wp == 0 else CONFIG["rate"] + (wp >= 6)
        for q in range(4):
            for pi, pair in enumerate(pairs_for(w0, w1)):
                if wp == 0:
                    drain_until((gQ[0] if q == 0 else gKb[q])[PREG[pi]])
                elif q == 0 and QBLK[wp] != QBLK[wp - 1]:
                    drain_until(gQ[QBLK[wp]][PREG[pi]])
                ph = scores_exp_q(pair, q)
                drive(rate)
                try_pops(2)
                pend.append((pi, pair, q, ph, pvd, wp))
    while pend:
        try_pops(4, minlag=0)
        drive(6)
    drain_all()


def build():
    if "nc" in _CACHED:
        return _CACHED["nc"]
    import concourse.tile as tile
    from concourse import bacc

    nc = bacc.Bacc("TRN2", target_bir_lowering=False, debug=False)
    with tile.TileContext(nc) as tc:
        with ExitStack() as ctx:
            _emit(ctx, tc)
    nc.compile()
    _CACHED["nc"] = nc
    return nc


def _split85(a):
    """f32 array -> (e4m3 main, e5m2 residual)."""
    hi = a.astype(ml_dtypes.float8_e4m3)
    lo = (a - hi.astype(np.float32)).astype(ml_dtypes.float8_e5m2)
    return hi, lo


def _act_layout(aT, nk):
    """[nk*128, S] -> [128, nk*S] with chunk-major free dim."""
    return np.ascontiguousarray(
        aT.reshape(nk, 128, S).transpose(1, 0, 2).reshape(128, nk * S))


def _w_layout(w, nk):
    """[nk*128, HD] -> [128, nk*HD]."""
    return np.ascontiguousarray(
        w.reshape(nk, 128, HD).transpose(1, 0, 2).reshape(128, nk * HD))


def make_in_maps(hidden_states, encoder_hidden_states, Wq, Wk, Wv, Wo):
    bf = ml_dtypes.bfloat16
    xs, encs = [], []
    for b in range(2):
        xT = np.ascontiguousarray(np.asarray(hidden_states[b], np.float32).T)
        x8, xr = _split85(xT)
        xs.append((_act_layout(x8, CK), _act_layout(xr, CK)))
        eT = np.ascontiguousarray(np.asarray(encoder_hidden_states[b], np.float32).T)
        e8, er = _split85(eT)
        encs.append((_act_layout(e8, CCK), _act_layout(er, CCK)))
    in_maps = []
    for core in range(8):
        b, g = divmod(core, 4)
        cols = slice(g * HD, (g + 1) * HD)
        wq8, wqr = _split85(np.ascontiguousarray(np.asarray(Wq[:, cols], np.float32)))
        wk8, wkr = _split85(np.ascontiguousarray(np.asarray(Wk[:, cols], np.float32)))
        wv8, wvr = _split85(np.ascontiguousarray(np.asarray(Wv[:, cols], np.float32)))
        in_maps.append({
            "x8": xs[b][0], "xr": xs[b][1],
            "enc8": encs[b][0], "encr": encs[b][1],
            "wq8": _w_layout(wq8, CK), "wqr": _w_layout(wqr, CK),
            "wk8": _w_layout(wk8, CCK), "wkr": _w_layout(wkr, CCK),
            "wv8": _w_layout(wv8, CCK), "wvr": _w_layout(wvr, CCK),
            "wo": np.ascontiguousarray(np.asarray(Wo[cols, :], np.float32)).astype(bf),
        })
    return in_maps


def kernel(hidden_states, encoder_hidden_states, Wq, Wk, Wv, Wo, b_o):
    from concourse.bass_utils import run_bass_kernel_spmd

    nc = build()
    in_maps = make_in_maps(hidden_states, encoder_hidden_states, Wq, Wk, Wv, Wo)
    res = run_bass_kernel_spmd(nc, in_maps, core_ids=list(range(8)))
    outs = [np.asarray(res.results[c]["out"], np.float32) for c in range(8)]
    full = np.stack([
        outs[0] + outs[1] + outs[2] + outs[3],
        outs[4] + outs[5] + outs[6] + outs[7],
    ]).astype(np.float32)
    full += np.asarray(b_o, np.float32)
    return full
